# revision 15
# baseline (speedup 1.0000x reference)
"""Trainium2 Bass kernel for per-edge bilinear scoring (GNN message passing).

Reference computation:
    src, tgt = edge_label_index            # [E], [E]
    logits[e] = x_source[src[e]] @ W @ x_target[tgt[e]] + b

Strategy (8 NeuronCores, SPMD single program, per-core data):
  - Host buckets edges by src//12500 -> core c owns edges whose src is in
    node chunk c; within a core edges are sorted by tgt.
  - Phase 1 (device): y = x_source[chunk] @ W for the core's 12544 rows
    (PE matmul), stored to a DRAM scratch table.
  - Phase 2 (device): blocks of 2048 edges; two dma_gather instructions per
    block (SWDGE "ant" bulk gather, 4 parallel descriptor-gen queues) pull
    the y rows and x_target rows into SBUF; DVE multiplies, ACT/DVE reduce
    per 128-edge tile into logits; bias added once at the end.
  - tgt indices exceed int16, so block b gathers from a sliding 32768-row
    window of x_target whose base is a fixed linear function of b
    (edges are tgt-sorted, so indices stay within the window; host asserts).
  - Host inverse-permutes the per-core outputs back to edge order.
"""

import contextlib
import os
import sys

for _p in ("/opt/trn_rl_repo",):
    if os.path.isdir(_p) and _p not in sys.path:
        sys.path.insert(0, _p)

import numpy as np

import concourse.bacc as bacc
import concourse.bass as bass
import concourse.mybir as mybir
from concourse.bass_utils import run_bass_kernel_spmd
from concourse.library_config import mlp

N = 100000
E = 2000000
D = 128
NCORES = 8
CH = N // NCORES            # 12500 node rows per core chunk
CHP = 12800                 # chunk rows padded to 25 groups of 4*128
NI = 1024                   # edges per gather block
NBLK = 248                  # gather blocks per core
CAP = NI * NBLK             # 253952 edge slots per core
CAP_T = CAP // 128          # 1984 edge tiles per core
WIN = 32768                 # sliding x_target window rows (int16 range)
MARGIN = 8192
NB = 8                      # gather buffer pairs
NQ = 4                      # SWDGE queues
ACT_RED_EVERY = 3           # every 3rd block reduced on ACT, rest on DVE

F32 = mybir.dt.float32
I16 = mybir.dt.int16

_NC_CACHE = None
_LAST_EXEC_NS = None
_TRACE = False


def _win_base(blk, cap, n_tab):
    if n_tab <= WIN:
        return 0
    b = (blk * NI + NI // 2) * n_tab // cap - WIN // 2
    return max(0, min(n_tab - WIN, b))


def _build_nc(cap=CAP, chp=CHP, n_tab=N, nblk=None, num_devices=NCORES):
    if nblk is None:
        nblk = cap // NI
    assert nblk * NI == cap
    cap_t = cap // 128
    n_xtiles = chp // 128
    scols = cap // 16

    nc = bacc.Bacc("TRN2", target_bir_lowering=False, debug=False,
                   num_devices=num_devices, num_swdge_queues=NQ)
    xsT = nc.dram_tensor("xsT", [D, chp], F32, kind="ExternalInput")
    xt_tab = nc.dram_tensor("xt_tab", [n_tab, D], F32, kind="ExternalInput")
    W = nc.dram_tensor("W", [D, D], F32, kind="ExternalInput")
    b_col = nc.dram_tensor("b_col", [D, 1], F32, kind="ExternalInput")
    src16 = nc.dram_tensor("src16", [128, scols], I16, kind="ExternalInput")
    tgt16 = nc.dram_tensor("tgt16", [128, scols], I16, kind="ExternalInput")
    out = nc.dram_tensor("out", [128, cap_t], F32, kind="ExternalOutput")
    y_tab = nc.dram_tensor("y_tab", [chp, D], F32, kind="ExternalOutput")

    with contextlib.ExitStack() as ctx:
        en = ctx.enter_context
        # SBUF tensors
        W_sb = en(nc.sbuf_tensor("W_sb", [D, D], F32))
        b_sb = en(nc.sbuf_tensor("b_sb", [D, 1], F32))
        src_sb = en(nc.sbuf_tensor("src_sb", [128, scols], I16))
        tgt_sb = en(nc.sbuf_tensor("tgt_sb", [128, scols], I16))
        logits = en(nc.sbuf_tensor("logits", [128, cap_t], F32))
        xs_t = [en(nc.sbuf_tensor(f"xs_t{i}", [D, 512], F32)) for i in range(2)]
        y_sb = [en(nc.sbuf_tensor(f"y_sb{i}", [128, 4, D], F32))
                for i in range(2)]
        y_ps = [en(nc.psum_tensor(f"y_ps{i}", [128, D], F32)) for i in range(4)]
        ys_buf = [en(nc.sbuf_tensor(f"ys_buf{i}", [128, NI // 128, D], F32))
                  for i in range(NB)]
        xt_buf = [en(nc.sbuf_tensor(f"xt_buf{i}", [128, NI // 128, D], F32))
                  for i in range(NB)]
        prod = [en(nc.sbuf_tensor(f"prod{i}", [128, NI // 128, D], F32))
                for i in range(2)]
        # semaphores
        ld = en(nc.semaphore("ld"))        # W/b/src/tgt loads (16 each)
        xld = [en(nc.semaphore(f"xld{i}")) for i in range(2)]
        mm = en(nc.semaphore("mm"))        # matmuls done (1 each)
        cp = en(nc.semaphore("cp"))        # psum->sbuf copies (1 each)
        yst = [en(nc.semaphore(f"yst{i}")) for i in range(2)]
        g_ys = [en(nc.semaphore(f"gys{i}")) for i in range(NB)]  # ys gathers
        g_xt = [en(nc.semaphore(f"gxt{i}")) for i in range(NB)]  # xt gathers
        mul = en(nc.semaphore("mul"))      # block products (1 each)
        reda = en(nc.semaphore("reda"))    # ACT block reductions (1 each)
        redv = en(nc.semaphore("redv"))    # DVE block reductions (1 each)
        bias = en(nc.semaphore("bias"))    # bias added (1)
        od = en(nc.semaphore("od"))        # out store (16)

        def is_dve_blk(b):
            return b % ACT_RED_EVERY != 0

        def act_count_through(b):
            # number of ACT-reduced blocks among 0..b
            return (b + 1) - sum(1 for x in range(b + 1) if is_dve_blk(x))

        with nc.Block() as block:

            @block.sync
            def _(sync):
                sync.dma_start(out=W_sb[:], in_=W[:]).then_inc(ld, 16)
                sync.dma_start(out=b_sb[:], in_=b_col[:]).then_inc(ld, 16)
                sync.dma_start(out=src_sb[:], in_=src16[:]).then_inc(ld, 16)
                sync.dma_start(out=tgt_sb[:], in_=tgt16[:]).then_inc(ld, 16)
                # phase 1: groups of 4 tiles (512 rows per DMA)
                ngrp = n_xtiles // 4
                for j in range(min(2, ngrp)):
                    sync.dma_start(out=xs_t[j][:],
                                   in_=xsT[:, j * 512:(j + 1) * 512]
                                   ).then_inc(xld[j], 16)
                for j in range(ngrp):
                    if j + 2 < ngrp:
                        sync.wait_ge(mm, 4 * (j + 1))
                        sync.dma_start(
                            out=xs_t[j % 2][:],
                            in_=xsT[:, (j + 2) * 512:(j + 3) * 512],
                        ).then_inc(xld[j % 2], 16)
                    sync.wait_ge(cp, 4 * (j + 1))
                    sync.dma_start(
                        out=y_tab[j * 512:(j + 1) * 512, :].rearrange(
                            "(g p) d -> p g d", p=128),
                        in_=y_sb[j % 2][:]).then_inc(yst[j % 2], 16)
                # final output
                sync.wait_ge(bias, 1)
                sync.dma_start(out=out[:], in_=logits[:]).then_inc(od, 16)
                sync.wait_ge(od, 16)

            @block.tensor
            def _(tensor):
                tensor.wait_ge(ld, 64)           # all const loads done
                for i in range(n_xtiles):
                    j = i // 4
                    tensor.wait_ge(xld[j % 2], 16 * (j // 2 + 1))
                    if i >= 4:
                        tensor.wait_ge(cp, i - 3)
                    tensor.matmul(
                        out=y_ps[i % 4][:],
                        lhsT=xs_t[j % 2][:, (i % 4) * 128:(i % 4 + 1) * 128],
                        rhs=W_sb[:], start=True,
                        stop=True).then_inc(mm, 1)

            @block.vector
            def _(vector):
                # phase 1: psum -> sbuf copies
                for i in range(n_xtiles):
                    j = i // 4
                    vector.wait_ge(mm, i + 1)
                    if j >= 2 and i % 4 == 0:
                        vector.wait_ge(yst[j % 2], 16 * (j // 2))
                    vector.tensor_copy(out=y_sb[j % 2][:, i % 4, :],
                                       in_=y_ps[i % 4][:]).then_inc(cp, 1)
                # phase 2: block products (+ some reductions)
                for blk in range(nblk):
                    k = blk % NB
                    vector.wait_ge(g_ys[k], 16 * (blk // NB + 1))
                    vector.wait_ge(g_xt[k], 16 * (blk // NB + 1))
                    if blk >= 2 and not is_dve_blk(blk - 2):
                        # prod[blk%2] reuse: ACT must be done with blk-2
                        # (DVE-reduced blocks are finished in-stream)
                        vector.wait_ge(reda, act_count_through(blk - 2))
                    vector.tensor_tensor(
                        out=prod[blk % 2][:], in0=ys_buf[k][:],
                        in1=xt_buf[k][:],
                        op=mybir.AluOpType.mult).then_inc(mul, 1)
                    if is_dve_blk(blk):
                        nt = NI // 128
                        vector.wait_ge(mul, blk + 1)
                        vector.tensor_reduce(
                            out=logits[:, blk * nt:(blk + 1) * nt],
                            in_=prod[blk % 2][:], axis=mybir.AxisListType.X,
                            op=mybir.AluOpType.add).then_inc(redv, 1)
                # bias
                vector.wait_ge(reda, act_count_through(nblk - 1))
                vector.wait_ge(redv, nblk - act_count_through(nblk - 1))
                vector.tensor_scalar_add(out=logits[:], in0=logits[:],
                                         scalar1=b_sb[:, :1]).then_inc(bias, 1)

            @block.scalar
            def _(scalar):
                nt = NI // 128
                for blk in range(nblk):
                    if is_dve_blk(blk):
                        continue
                    scalar.wait_ge(mul, blk + 1)
                    for k in range(nt):
                        t = blk * nt + k
                        ins = scalar.activation(
                            out=prod[blk % 2][:, k, :],
                            in_=prod[blk % 2][:, k, :],
                            func=mybir.ActivationFunctionType.Copy,
                            accum_out=logits[:, t:t + 1],
                        )
                    ins.then_inc(reda, 1)

            @block.gpsimd
            def _(gpsimd):
                gpsimd.load_library(mlp)
                gpsimd.wait_ge(ld, 64)
                ngrp = n_xtiles // 4
                pre = min(6, NB - 1, nblk)

                def gather_xt(blk):
                    k = blk % NB
                    c0 = blk * (NI // 16)
                    c1 = (blk + 1) * (NI // 16)
                    base = _win_base(blk, cap, n_tab)
                    gpsimd.dma_gather(
                        xt_buf[k][:], xt_tab[base:base + min(WIN, n_tab), :],
                        tgt_sb[:, c0:c1],
                        NI, NI, D, single_packet=False,
                        queue_num=(k + 1) % NQ,
                    ).then_inc(g_xt[k], 16)

                # prologue: xt gathers don't need y_tab -> overlap phase 1
                for blk in range(pre):
                    gather_xt(blk)
                gpsimd.wait_ge(yst[0], 16 * ((ngrp + 1) // 2))
                gpsimd.wait_ge(yst[1], 16 * (ngrp // 2))
                for blk in range(nblk):
                    k = blk % NB
                    if blk >= NB:
                        gpsimd.wait_ge(mul, blk - NB + 1)
                    c0 = blk * (NI // 16)
                    c1 = (blk + 1) * (NI // 16)
                    gpsimd.dma_gather(
                        ys_buf[k][:], y_tab[:], src_sb[:, c0:c1],
                        NI, NI, D, single_packet=False,
                        queue_num=k % NQ,
                    ).then_inc(g_ys[k], 16)
                    nxt = blk + pre
                    if nxt < nblk:
                        if nxt >= NB:
                            gpsimd.wait_ge(mul, nxt - NB + 1)
                        gather_xt(nxt)

    nc.compile()
    return nc


def _get_nc():
    global _NC_CACHE
    if _NC_CACHE is None:
        _NC_CACHE = _build_nc()
    return _NC_CACHE


def _idx16_wrap(idx, cap):
    """[cap] int array -> [128, cap//16] int16 in dma_gather layout.

    Within each NI-block, index j lives at partition j%16, column j//16;
    block b occupies columns [b*NI//16, (b+1)*NI//16); rows replicated x8.
    """
    nblk = cap // NI
    a = idx.astype(np.int16).reshape(nblk, NI // 16, 16)
    c = a.transpose(0, 2, 1)              # [nblk, 16, NI//16]
    c = np.concatenate(list(c), axis=1)   # [16, nblk*NI//16]
    return np.ascontiguousarray(np.tile(c, (8, 1)))


def kernel(x_source, x_target, edge_label_index, W, b):
    global _LAST_EXEC_NS
    x_source = np.asarray(x_source, dtype=np.float32)
    x_target = np.asarray(x_target, dtype=np.float32)
    eli = np.asarray(edge_label_index)
    W = np.asarray(W, dtype=np.float32)
    bval = float(np.asarray(b))

    src = eli[0].astype(np.int64)
    tgt = eli[1].astype(np.int64)
    n_edges = src.shape[0]

    core_of = src // CH
    perm = np.lexsort((tgt, core_of))
    counts = np.bincount(core_of, minlength=NCORES)
    assert counts.max() <= CAP, f"bucket overflow: {counts.max()} > {CAP}"

    in_maps = []
    perms = []
    bounds = np.concatenate(([0], np.cumsum(counts)))
    b_colv = np.full((D, 1), bval, dtype=np.float32)
    bases = np.array([_win_base(blk, CAP, N) for blk in range(NBLK)])
    for c in range(NCORES):
        pc = perm[bounds[c]:bounds[c + 1]]
        perms.append(pc)
        srcI = np.zeros(CAP, dtype=np.int64)
        tgtI = np.zeros(CAP, dtype=np.int64)
        srcI[:pc.size] = src[pc] - c * CH
        tgtI[:pc.size] = tgt[pc]
        tgtI[pc.size:] = tgtI[max(pc.size - 1, 0)]   # pad with last (sorted)
        # rebase tgt by per-block window base
        tgtR = tgtI.reshape(NBLK, NI) - bases[:, None]
        assert tgtR.min() >= 0 and tgtR.max() < WIN, \
            f"window overflow core {c}: {tgtR.min()} {tgtR.max()}"
        xsT_c = np.zeros((D, CHP), dtype=np.float32)
        xsT_c[:, :CH] = x_source[c * CH:(c + 1) * CH].T
        in_maps.append({
            "xsT": np.ascontiguousarray(xsT_c),
            "xt_tab": x_target,
            "W": W,
            "b_col": b_colv,
            "src16": _idx16_wrap(srcI, CAP),
            "tgt16": _idx16_wrap(tgtR.reshape(-1), CAP),
        })

    nc = _get_nc()
    res = run_bass_kernel_spmd(nc, in_maps, core_ids=list(range(NCORES)),
                               trace=_TRACE)
    _LAST_EXEC_NS = res.exec_time_ns

    result = np.empty(n_edges, dtype=np.float32)
    for c in range(NCORES):
        out_c = res.results[c]["out"]            # [128, CAP_T]
        logits_c = out_c.T.reshape(-1)[:perms[c].size]
        result[perms[c]] = logits_c
    return result


# revision 16
# speedup vs baseline: 1.1044x; 1.1044x over previous
"""Trainium2 Bass kernel for per-edge bilinear scoring (GNN message passing).

Reference computation:
    src, tgt = edge_label_index            # [E], [E]
    logits[e] = x_source[src[e]] @ W @ x_target[tgt[e]] + b

Strategy (8 NeuronCores, SPMD single program, per-core data):
  - Host buckets edges by src//12500 -> core c owns edges whose src is in
    node chunk c; within a core edges are sorted by tgt.
  - Phase 1 (device): y = x_source[chunk] @ W for the core's 12544 rows
    (PE matmul), stored to a DRAM scratch table.
  - Phase 2 (device): blocks of 2048 edges; two dma_gather instructions per
    block (SWDGE "ant" bulk gather, 4 parallel descriptor-gen queues) pull
    the y rows and x_target rows into SBUF; DVE multiplies, ACT/DVE reduce
    per 128-edge tile into logits; bias added once at the end.
  - tgt indices exceed int16, so block b gathers from a sliding 32768-row
    window of x_target whose base is a fixed linear function of b
    (edges are tgt-sorted, so indices stay within the window; host asserts).
  - Host inverse-permutes the per-core outputs back to edge order.
"""

import contextlib
import os
import sys

for _p in ("/opt/trn_rl_repo",):
    if os.path.isdir(_p) and _p not in sys.path:
        sys.path.insert(0, _p)

import numpy as np

import concourse.bacc as bacc
import concourse.bass as bass
import concourse.mybir as mybir
from concourse.bass_utils import run_bass_kernel_spmd
from concourse.library_config import mlp

N = 100000
E = 2000000
D = 128
NCORES = 8
CH = N // NCORES            # 12500 node rows per core chunk
CHP = 12800                 # chunk rows padded to 25 groups of 4*128
NI = 1024                   # edges per gather block
NBLK = 248                  # gather blocks per core
CAP = NI * NBLK             # 253952 edge slots per core
CAP_T = CAP // 128          # 1984 edge tiles per core
WIN = 32768                 # sliding x_target window rows (int16 range)
MARGIN = 8192
NB = 8                      # gather buffer pairs
NQ = 4                      # SWDGE queues
ACT_RED_EVERY = 3           # every 3rd block reduced on ACT, rest on DVE

F32 = mybir.dt.float32
I16 = mybir.dt.int16

_NC_CACHE = None
_LAST_EXEC_NS = None
_TRACE = False


def _win_base(blk, cap, n_tab):
    if n_tab <= WIN:
        return 0
    b = (blk * NI + NI // 2) * n_tab // cap - WIN // 2
    return max(0, min(n_tab - WIN, b))


def _build_nc(cap=CAP, chp=CHP, n_tab=N, nblk=None, num_devices=NCORES):
    if nblk is None:
        nblk = cap // NI
    assert nblk * NI == cap
    cap_t = cap // 128
    n_xtiles = chp // 128
    scols = cap // 16

    nc = bacc.Bacc("TRN2", target_bir_lowering=False, debug=False,
                   num_devices=num_devices, num_swdge_queues=NQ)
    xsT = nc.dram_tensor("xsT", [D, chp], F32, kind="ExternalInput")
    xt_tab = nc.dram_tensor("xt_tab", [n_tab, D], F32, kind="ExternalInput")
    W = nc.dram_tensor("W", [D, D], F32, kind="ExternalInput")
    b_col = nc.dram_tensor("b_col", [D, 1], F32, kind="ExternalInput")
    src16 = nc.dram_tensor("src16", [128, scols], I16, kind="ExternalInput")
    tgt16 = nc.dram_tensor("tgt16", [128, scols], I16, kind="ExternalInput")
    out = nc.dram_tensor("out", [128, cap_t], F32, kind="ExternalOutput")
    y_tab = nc.dram_tensor("y_tab", [chp, D], F32, kind="ExternalOutput")

    with contextlib.ExitStack() as ctx:
        en = ctx.enter_context
        # SBUF tensors
        W_sb = en(nc.sbuf_tensor("W_sb", [D, D], F32))
        b_sb = en(nc.sbuf_tensor("b_sb", [D, 1], F32))
        src_sb = en(nc.sbuf_tensor("src_sb", [128, scols], I16))
        tgt_sb = en(nc.sbuf_tensor("tgt_sb", [128, scols], I16))
        logits = en(nc.sbuf_tensor("logits", [128, cap_t], F32))
        xs_t = [en(nc.sbuf_tensor(f"xs_t{i}", [D, 512], F32)) for i in range(2)]
        y_sb = [en(nc.sbuf_tensor(f"y_sb{i}", [128, 4, D], F32))
                for i in range(2)]
        y_ps = [en(nc.psum_tensor(f"y_ps{i}", [128, D], F32)) for i in range(4)]
        ys_buf = [en(nc.sbuf_tensor(f"ys_buf{i}", [128, NI // 128, D], F32))
                  for i in range(NB)]
        xt_buf = [en(nc.sbuf_tensor(f"xt_buf{i}", [128, NI // 128, D], F32))
                  for i in range(NB)]
        prod = [en(nc.sbuf_tensor(f"prod{i}", [128, NI // 128, D], F32))
                for i in range(2)]
        # semaphores
        ld = en(nc.semaphore("ld"))        # W/b/src/tgt loads (16 each)
        xld = [en(nc.semaphore(f"xld{i}")) for i in range(2)]
        mm = en(nc.semaphore("mm"))        # matmuls done (1 each)
        cp = en(nc.semaphore("cp"))        # psum->sbuf copies (1 each)
        yst = [en(nc.semaphore(f"yst{i}")) for i in range(2)]
        g_ys = [en(nc.semaphore(f"gys{i}")) for i in range(NB)]  # ys gathers
        g_xt = [en(nc.semaphore(f"gxt{i}")) for i in range(NB)]  # xt gathers
        mul = en(nc.semaphore("mul"))      # block products (1 each)
        reda = en(nc.semaphore("reda"))    # ACT block reductions (1 each)
        redv = en(nc.semaphore("redv"))    # DVE block reductions (1 each)
        bias = en(nc.semaphore("bias"))    # bias added (1)
        od = en(nc.semaphore("od"))        # out store (16)

        def is_dve_blk(b):
            return b % ACT_RED_EVERY != 0

        def act_count_through(b):
            # number of ACT-reduced blocks among 0..b
            return (b + 1) - sum(1 for x in range(b + 1) if is_dve_blk(x))

        with nc.Block() as block:

            @block.sync
            def _(sync):
                sync.dma_start(out=W_sb[:], in_=W[:]).then_inc(ld, 16)
                sync.dma_start(out=b_sb[:], in_=b_col[:]).then_inc(ld, 16)
                sync.dma_start(out=src_sb[:], in_=src16[:]).then_inc(ld, 16)
                sync.dma_start(out=tgt_sb[:], in_=tgt16[:]).then_inc(ld, 16)
                # phase 1: groups of 4 tiles (512 rows per DMA)
                ngrp = n_xtiles // 4
                for j in range(min(2, ngrp)):
                    sync.dma_start(out=xs_t[j][:],
                                   in_=xsT[:, j * 512:(j + 1) * 512]
                                   ).then_inc(xld[j], 16)
                for j in range(ngrp):
                    if j + 2 < ngrp:
                        sync.wait_ge(mm, 4 * (j + 1))
                        sync.dma_start(
                            out=xs_t[j % 2][:],
                            in_=xsT[:, (j + 2) * 512:(j + 3) * 512],
                        ).then_inc(xld[j % 2], 16)
                    sync.wait_ge(cp, 4 * (j + 1))
                    sync.dma_start(
                        out=y_tab[j * 512:(j + 1) * 512, :].rearrange(
                            "(g p) d -> p g d", p=128),
                        in_=y_sb[j % 2][:]).then_inc(yst[j % 2], 16)
                # final output
                sync.wait_ge(bias, 1)
                sync.dma_start(out=out[:], in_=logits[:]).then_inc(od, 16)
                sync.wait_ge(od, 16)

            @block.tensor
            def _(tensor):
                tensor.wait_ge(ld, 64)           # all const loads done
                for i in range(n_xtiles):
                    j = i // 4
                    tensor.wait_ge(xld[j % 2], 16 * (j // 2 + 1))
                    if i >= 4:
                        tensor.wait_ge(cp, i - 3)
                    tensor.matmul(
                        out=y_ps[i % 4][:],
                        lhsT=xs_t[j % 2][:, (i % 4) * 128:(i % 4 + 1) * 128],
                        rhs=W_sb[:], start=True,
                        stop=True).then_inc(mm, 1)

            @block.vector
            def _(vector):
                # phase 1: psum -> sbuf copies
                for i in range(n_xtiles):
                    j = i // 4
                    vector.wait_ge(mm, i + 1)
                    if j >= 2 and i % 4 == 0:
                        vector.wait_ge(yst[j % 2], 16 * (j // 2))
                    vector.tensor_copy(out=y_sb[j % 2][:, i % 4, :],
                                       in_=y_ps[i % 4][:]).then_inc(cp, 1)
                # phase 2: block products (+ some reductions)
                for blk in range(nblk):
                    k = blk % NB
                    vector.wait_ge(g_ys[k], 16 * (blk // NB + 1))
                    vector.wait_ge(g_xt[k], 16 * (blk // NB + 1))
                    if blk >= 2 and not is_dve_blk(blk - 2):
                        # prod[blk%2] reuse: ACT must be done with blk-2
                        # (DVE-reduced blocks are finished in-stream)
                        vector.wait_ge(reda, act_count_through(blk - 2))
                    vector.tensor_tensor(
                        out=prod[blk % 2][:], in0=ys_buf[k][:],
                        in1=xt_buf[k][:],
                        op=mybir.AluOpType.mult).then_inc(mul, 1)
                    if is_dve_blk(blk):
                        nt = NI // 128
                        vector.wait_ge(mul, blk + 1)
                        vector.tensor_reduce(
                            out=logits[:, blk * nt:(blk + 1) * nt],
                            in_=prod[blk % 2][:], axis=mybir.AxisListType.X,
                            op=mybir.AluOpType.add).then_inc(redv, 1)
                # bias
                vector.wait_ge(reda, act_count_through(nblk - 1))
                vector.wait_ge(redv, nblk - act_count_through(nblk - 1))
                vector.tensor_scalar_add(out=logits[:], in0=logits[:],
                                         scalar1=b_sb[:, :1]).then_inc(bias, 1)

            @block.scalar
            def _(scalar):
                nt = NI // 128
                for blk in range(nblk):
                    if is_dve_blk(blk):
                        continue
                    scalar.wait_ge(mul, blk + 1)
                    for k in range(nt):
                        t = blk * nt + k
                        ins = scalar.activation(
                            out=prod[blk % 2][:, k, :],
                            in_=prod[blk % 2][:, k, :],
                            func=mybir.ActivationFunctionType.Copy,
                            accum_out=logits[:, t:t + 1],
                        )
                    ins.then_inc(reda, 1)

            @block.gpsimd
            def _(gpsimd):
                gpsimd.load_library(mlp)
                gpsimd.wait_ge(ld, 64)
                ngrp = n_xtiles // 4
                gpsimd.wait_ge(yst[0], 16 * ((ngrp + 1) // 2))
                gpsimd.wait_ge(yst[1], 16 * (ngrp // 2))
                for blk in range(nblk):
                    k = blk % NB
                    if blk >= NB:
                        gpsimd.wait_ge(mul, blk - NB + 1)
                    c0 = blk * (NI // 16)
                    c1 = (blk + 1) * (NI // 16)
                    gpsimd.dma_gather(
                        ys_buf[k][:], y_tab[:], src_sb[:, c0:c1],
                        NI, NI, D, single_packet=False,
                        queue_num=k % NQ,
                    ).then_inc(g_ys[k], 16)
                    base = _win_base(blk, cap, n_tab)
                    gpsimd.dma_gather(
                        xt_buf[k][:], xt_tab[base:base + min(WIN, n_tab), :],
                        tgt_sb[:, c0:c1],
                        NI, NI, D, single_packet=False,
                        queue_num=(k + 1) % NQ,
                    ).then_inc(g_xt[k], 16)

    nc.compile()
    return nc


def _get_nc():
    global _NC_CACHE
    if _NC_CACHE is None:
        _NC_CACHE = _build_nc()
    return _NC_CACHE


def _idx16_wrap(idx, cap):
    """[cap] int array -> [128, cap//16] int16 in dma_gather layout.

    Within each NI-block, index j lives at partition j%16, column j//16;
    block b occupies columns [b*NI//16, (b+1)*NI//16); rows replicated x8.
    """
    nblk = cap // NI
    a = idx.astype(np.int16).reshape(nblk, NI // 16, 16)
    c = a.transpose(0, 2, 1)              # [nblk, 16, NI//16]
    c = np.concatenate(list(c), axis=1)   # [16, nblk*NI//16]
    return np.ascontiguousarray(np.tile(c, (8, 1)))


def kernel(x_source, x_target, edge_label_index, W, b):
    global _LAST_EXEC_NS
    x_source = np.asarray(x_source, dtype=np.float32)
    x_target = np.asarray(x_target, dtype=np.float32)
    eli = np.asarray(edge_label_index)
    W = np.asarray(W, dtype=np.float32)
    bval = float(np.asarray(b))

    src = eli[0].astype(np.int64)
    tgt = eli[1].astype(np.int64)
    n_edges = src.shape[0]

    core_of = src // CH
    perm = np.lexsort((tgt, core_of))
    counts = np.bincount(core_of, minlength=NCORES)
    assert counts.max() <= CAP, f"bucket overflow: {counts.max()} > {CAP}"

    in_maps = []
    perms = []
    bounds = np.concatenate(([0], np.cumsum(counts)))
    b_colv = np.full((D, 1), bval, dtype=np.float32)
    bases = np.array([_win_base(blk, CAP, N) for blk in range(NBLK)])
    for c in range(NCORES):
        pc = perm[bounds[c]:bounds[c + 1]]
        perms.append(pc)
        srcI = np.zeros(CAP, dtype=np.int64)
        tgtI = np.zeros(CAP, dtype=np.int64)
        srcI[:pc.size] = src[pc] - c * CH
        tgtI[:pc.size] = tgt[pc]
        tgtI[pc.size:] = tgtI[max(pc.size - 1, 0)]   # pad with last (sorted)
        # rebase tgt by per-block window base
        tgtR = tgtI.reshape(NBLK, NI) - bases[:, None]
        assert tgtR.min() >= 0 and tgtR.max() < WIN, \
            f"window overflow core {c}: {tgtR.min()} {tgtR.max()}"
        xsT_c = np.zeros((D, CHP), dtype=np.float32)
        xsT_c[:, :CH] = x_source[c * CH:(c + 1) * CH].T
        in_maps.append({
            "xsT": np.ascontiguousarray(xsT_c),
            "xt_tab": x_target,
            "W": W,
            "b_col": b_colv,
            "src16": _idx16_wrap(srcI, CAP),
            "tgt16": _idx16_wrap(tgtR.reshape(-1), CAP),
        })

    nc = _get_nc()
    res = run_bass_kernel_spmd(nc, in_maps, core_ids=list(range(NCORES)),
                               trace=_TRACE)
    _LAST_EXEC_NS = res.exec_time_ns

    result = np.empty(n_edges, dtype=np.float32)
    for c in range(NCORES):
        out_c = res.results[c]["out"]            # [128, CAP_T]
        logits_c = out_c.T.reshape(-1)[:perms[c].size]
        result[perms[c]] = logits_c
    return result
